# revision 33
# baseline (speedup 1.0000x reference)
"""Trainium2 Bass kernel for batched cross-attention + multiscale sigmoid gate.

Reference computation (per batch b):
    q = x1 @ Wq.T + bq ; k = x2 @ Wk.T + bk ; v = x2 @ Wv.T + bv
    attn = softmax(q @ k.T, axis=-1)              (unscaled)
    out = attn @ v
    s = out @ (W1+W2+W3).T + (b1+b2+b3)
    out = out * sigmoid(s)
    return gamma * out + x1

Strategy: pure data-parallel over batch (16 batches -> 8 cores x 2),
no collectives. Everything on-chip is kept transposed ([feature, token])
so all matmuls contract over the partition dim with zero on-device
transposes. ALL matmuls run fp8e4m3 DoubleRow (2x contraction tiles per
matmul instruction): Q/K/V projections, QK^T energy, PV, and the gate.
Weights are pre-scaled x32 on the host so their fp8 encodings stay out
of the subnormal range; every descale folds into an existing epilogue
(exp scale=1/1024, PV-drain scale=1/32, sigmoid scale=1/32) so the
rescaling is free. Softmax: fixed shift 64 (no row max), exp on ScalarE
-> bf16 P, denominator accumulated on VectorE (bf16 2x mode),
partition-all-reduced on GpSimd, reciprocal on VectorE, P normalized +
cast to fp8 split across VectorE/GpSimd. Per i-block the PV+gate of the
previous block is software-pipelined into the shadow of the current
block's softmax pipeline so TensorE never waits. Epilogues are spread
across ScalarE (Q/K drains, exp, sigmoid), GpSimd (V drain, PV drain),
and VectorE (den, norm, final gating) to keep every engine under the
fp8 TensorE roofline (~218us/core). Output is written bf16; residual
add (+x1) on the host.

Numerics (host-validated): full-output rel err 4.6e-3 vs f32 reference
(budget 2e-2). gamma ~ -0.063 scales the attention path to ~3% of the
output norm, so fp8 energy (abs energy noise ~0.5 pre-softmax) is safe.
"""

import math

import numpy as np
import ml_dtypes

import concourse.tile as tile
from concourse import mybir, bacc
from concourse.bass_isa import ReduceOp

P = 128
F32 = mybir.dt.float32
BF16 = mybir.dt.bfloat16
F8 = mybir.dt.float8e4
AF = mybir.ActivationFunctionType
OP = mybir.AluOpType
DR = mybir.MatmulPerfMode.DoubleRow

# full problem shape (hardcoded per harness contract)
B_FULL, N_FULL, D_FULL = 16, 2048, 1024
N_CORES = 8
SHIFT = 64.0
WSC = 32.0            # host-side weight scale (fp8 subnormal dodge)


def build(BPC, N, D, gamma, shift=SHIFT, reps=1):
    """Build the per-core Bass graph. BPC = batches per core."""
    DC = D // P          # feature chunks of 128
    NJ = N // P          # key tiles of 128
    PB = min(512, N)     # projection n-block
    NPB = N // PB
    IB = min(512, N)     # attention i-block (query block)
    NIB = N // IB
    KH = math.ceil(D / 512)  # V-projection k halves
    assert DC % 2 == 0 and NJ % 2 == 0

    nc = bacc.Bacc("TRN2", target_bir_lowering=False, debug=False,
                   num_devices=N_CORES)

    x1t_d = nc.declare_dram_parameter("x1t_8", [BPC, D, N], F8, isOutput=False)
    x2t_d = nc.declare_dram_parameter("x2t_8", [BPC, D, N], F8, isOutput=False)
    wq_d = nc.declare_dram_parameter("wq_t", [D, D], F8, isOutput=False)
    wk_d = nc.declare_dram_parameter("wk_t", [D, D], F8, isOutput=False)
    wv_d = nc.declare_dram_parameter("wv_t", [D, D], F8, isOutput=False)
    ws_d = nc.declare_dram_parameter("ws_t", [D, D], F8, isOutput=False)
    # all four bias vectors packed: one DMA dispatch instead of four
    ball_d = nc.declare_dram_parameter("b_all", [P, 4, DC], F32, isOutput=False)
    out_ext = nc.declare_dram_parameter("out", [BPC, D, N], BF16, isOutput=True)

    def r3(ap):  # [D, N] dram view -> [p, dc, n]
        return ap.rearrange("(c p) n -> p c n", p=P)

    def mm8(pst, lhsT3, rhs3, start, stop):
        """fp8 DoubleRow matmul over 2 contraction chunk-tiles."""
        nc.tensor.matmul(pst, lhsT=lhsT3, rhs=rhs3, start=start, stop=stop,
                         perf_mode=DR)

    with tile.TileContext(nc) as tc:
        with (
            tc.tile_pool(name="w8", bufs=1) as w8,
            tc.tile_pool(name="consts", bufs=1) as consts,
            tc.tile_pool(name="xin", bufs=4) as xin,
            tc.tile_pool(name="kv", bufs=1) as kvpool,
            tc.tile_pool(name="pall", bufs=2) as pall_pool,
            tc.tile_pool(name="p8", bufs=2) as p8_pool,
            tc.tile_pool(name="obf", bufs=2) as obf_pool,
            tc.tile_pool(name="small", bufs=1) as small,
            tc.tile_pool(name="gp", bufs=6) as gpool,
            tc.tile_pool(name="fin", bufs=3) as finpool,
            tc.tile_pool(name="psE", bufs=2, space="PSUM") as psE,
            tc.tile_pool(name="ps", bufs=4, space="PSUM") as ps,
        ):
            # constants / biases
            negshift = consts.tile([P, 1], F32)
            nc.vector.memset(negshift[:], -shift)

            # weights: fp8, resident for the whole kernel (batch-invariant).
            # Startup latency is dominated by serial DMA dispatch (~565ns
            # each), so use few, large DMAs and order them so the first Q
            # matmul group's operands land first: wq halves, x1 tile, x2
            # tile, biases. wk/wv/ws are dispatched between the first
            # projection groups.
            wq_sb = w8.tile([P, DC, D], F8, tag="wq")
            wk_sb = w8.tile([P, DC, D], F8, tag="wk")
            wv_sb = w8.tile([P, DC, D], F8, tag="wv")
            ws_sb = w8.tile([P, DC, D], F8, tag="ws")
            H = DC // 2
            nc.sync.dma_start(wq_sb[:, :H], r3(wq_d.ap())[:, :H])
            nc.sync.dma_start(wq_sb[:, H:], r3(wq_d.ap())[:, H:])
            nsl0 = slice(0, PB)
            x1t0 = xin.tile([P, DC, PB], F8, tag="xin")
            x2t0 = xin.tile([P, DC, PB], F8, tag="xin")
            nc.sync.dma_start(x1t0[:], r3(x1t_d[0])[:, :, nsl0])
            nc.sync.dma_start(x2t0[:], r3(x2t_d[0])[:, :, nsl0])
            b_all = consts.tile([P, 4, DC], F32)
            nc.sync.dma_start(b_all[:], ball_d[:])
            
            def emit_pv(p_lo, v_cur):
                # out = (P @ V')/32 + bv; drain alternates ACT/DVE (the
                # attention steady state is vector-drain-limited, not
                # matmul-limited)
                out_lo = obf_pool.tile([P, DC, IB], F8, tag="obf")
                for kc in range(DC):
                    o_ps = ps.tile([P, IB], F32, tag="ps")
                    for jp in range(NJ // 2):
                        mm8(o_ps[:],
                            v_cur[:, 2 * jp:2 * jp + 2, kc * P:(kc + 1) * P],
                            p_lo[:, 2 * jp:2 * jp + 2, :],
                            start=(jp == 0), stop=(jp == NJ // 2 - 1))
                    nc.vector.tensor_scalar(
                        out_lo[:, kc, :], o_ps[:], 1.0 / WSC,
                        b_all[:, 2, kc:kc + 1], OP.mult, OP.add)
                return out_lo

            def gate_final(out_lo, b_o, ib):
                isl = slice(ib * IB, (ib + 1) * IB)
                for ec in range(DC):
                    g_ps = ps.tile([P, IB], F32, tag="ps")
                    for dc2 in range(DC // 2):
                        mm8(g_ps[:],
                            ws_sb[:, 2 * dc2:2 * dc2 + 2, ec * P:(ec + 1) * P],
                            out_lo[:, 2 * dc2:2 * dc2 + 2, :],
                            start=(dc2 == 0), stop=(dc2 == DC // 2 - 1))
                    # sigmoid(x) = 0.5*tanh(x/2) + 0.5 ; Tanh shares the
                    # ACT table with Exp/Identity, so no table reloads.
                    # bs_r is bs/2 (host-prepped); fin = (tanh+1)*out and
                    # the remaining gamma/2 factor is applied on the host.
                    g_sb = gpool.tile([P, IB], BF16, tag="g")
                    nc.scalar.activation(g_sb[:], g_ps[:], AF.Tanh,
                                         bias=b_all[:, 3, ec:ec + 1],
                                         scale=1.0 / (2.0 * WSC))
                    fin = finpool.tile([P, IB], BF16, tag="fin")
                    nc.vector.scalar_tensor_tensor(
                        fin[:], g_sb[:], 1.0,
                        out_lo[:, ec, :], OP.add, OP.mult)
                    nc.sync.dma_start(
                        out_ext[b_o, ec * P:(ec + 1) * P, isl], fin[:])

            def flush(deferred):
                # PV + gate of a pending i-block (one block, or one batch,
                # behind -- keeps TensorE fed while softmax latency drains)
                p_lo, b_o, ib, v_cur = deferred
                gate_final(emit_pv(p_lo, v_cur), b_o, ib)

            first = True
            deferred = None  # (p_lo, b, ib, v_sb) pending PV + gate
            blist = [bb for _ in range(reps) for bb in range(BPC)]
            for bi, b in enumerate(blist):
                last_batch = bi == len(blist) - 1
                # ---- phase 1: projections (all fp8 DoubleRow) ----
                qt_sb = kvpool.tile([P, DC, N], F8, tag="qt")
                kt_sb = kvpool.tile([P, DC, N], F8, tag="kt")
                v_sb = None  # allocated after the deferred flush (WAR order)

                for pb in range(NPB):
                    nsl = slice(pb * PB, (pb + 1) * PB)
                    if first and pb == 0:
                        x1t, x2t = x1t0, x2t0
                    else:
                        x1t = xin.tile([P, DC, PB], F8, tag="xin")
                        nc.sync.dma_start(x1t[:], r3(x1t_d[b])[:, :, nsl])
                        x2t = xin.tile([P, DC, PB], F8, tag="xin")
                        nc.sync.dma_start(x2t[:], r3(x2t_d[b])[:, :, nsl])

                    # QT chunk [e, n] (SBUF resident fp8; stays x32 scaled)
                    for ec in range(DC):
                        pst = ps.tile([P, PB], F32, tag="ps")
                        for dc2 in range(DC // 2):
                            mm8(pst[:],
                                wq_sb[:, 2 * dc2:2 * dc2 + 2, ec * P:(ec + 1) * P],
                                x1t[:, 2 * dc2:2 * dc2 + 2, :],
                                start=(dc2 == 0), stop=(dc2 == DC // 2 - 1))
                        nc.scalar.activation(qt_sb[:, ec, nsl], pst[:], AF.Identity,
                                             bias=b_all[:, 0, ec:ec + 1])

                    if first and pb == 0:
                        for dc in range(DC):
                            nc.sync.dma_start(wk_sb[:, dc], r3(wk_d.ap())[:, dc])

                    # KT chunk (resident fp8, x32 scaled)
                    for ec in range(DC):
                        pst = ps.tile([P, PB], F32, tag="ps")
                        for dc2 in range(DC // 2):
                            mm8(pst[:],
                                wk_sb[:, 2 * dc2:2 * dc2 + 2, ec * P:(ec + 1) * P],
                                x2t[:, 2 * dc2:2 * dc2 + 2, :],
                                start=(dc2 == 0), stop=(dc2 == DC // 2 - 1))
                        nc.scalar.activation(kt_sb[:, ec, nsl], pst[:], AF.Identity,
                                             bias=b_all[:, 1, ec:ec + 1])

                    if pb == 0:
                        if first:
                            for dc in range(DC):
                                nc.sync.dma_start(wv_sb[:, dc],
                                                  r3(wv_d.ap())[:, dc])
                        # previous batch's last i-block PV+gate lands here,
                        # inside the new batch's projection stream
                        if deferred is not None:
                            flush(deferred)
                            deferred = None
                        v_sb = kvpool.tile([P, NJ, D], F8, tag="v")

                    # V chunk (resident [j, k] fp8, x32 scaled, no bias;
                    # bv added at the PV drain). Drain on DVE.
                    for js in range(PB // P):
                        for kh in range(KH):
                            k0 = kh * 512
                            kw = min(512, D - k0)
                            pst = ps.tile([P, PB], F32, tag="ps")
                            for dc2 in range(DC // 2):
                                mm8(pst[:, :kw],
                                    x2t[:, 2 * dc2:2 * dc2 + 2, js * P:(js + 1) * P],
                                    wv_sb[:, 2 * dc2:2 * dc2 + 2, k0:k0 + kw],
                                    start=(dc2 == 0), stop=(dc2 == DC // 2 - 1))
                            nc.vector.tensor_copy(
                                v_sb[:, pb * (PB // P) + js, k0:k0 + kw],
                                pst[:, :kw])

                    if first and pb == 0:
                        nc.sync.dma_start(ws_sb[:], r3(ws_d.ap()))
                        first = False

                # ---- phase 2: attention + gate, per i-block ----
                for ib in range(NIB):
                    isl = slice(ib * IB, (ib + 1) * IB)
                    p_all = pall_pool.tile([P, NJ, IB], BF16, tag="pall")
                    den_a = small.tile([P, IB], BF16, tag="dena")

                    # pass A: energy (fp8 DR) into 2-bank PSUM tiles so a
                    # single exp covers two j-tiles (exp bias/scale are
                    # constant, so tiles can share one ACT op -- halves the
                    # ACT op count); denominator accumulated in bf16 on DVE
                    # (2x mode)
                    for jh in range(NJ // 2):
                        ps2 = psE.tile([P, 2, IB], F32, tag="ps2")
                        for h in (0, 1):
                            j = 2 * jh + h
                            for dc2 in range(DC // 2):
                                mm8(ps2[:, h],
                                    kt_sb[:, 2 * dc2:2 * dc2 + 2,
                                          j * P:(j + 1) * P],
                                    qt_sb[:, 2 * dc2:2 * dc2 + 2, isl],
                                    start=(dc2 == 0),
                                    stop=(dc2 == DC // 2 - 1))
                        nc.scalar.activation(p_all[:, 2 * jh:2 * jh + 2, :],
                                             ps2[:], AF.Exp,
                                             bias=negshift[:, 0:1],
                                             scale=1.0 / (WSC * WSC))
                        for h in (0, 1):
                            j = 2 * jh + h
                            if j == 0:
                                nc.vector.tensor_copy(den_a[:],
                                                      p_all[:, j, :])
                            else:
                                nc.vector.tensor_tensor(den_a[:], den_a[:],
                                                        p_all[:, j, :],
                                                        OP.add)

                    # partition all-reduce on GpSimd (sum + broadcast in one
                    # op), then reciprocal on DVE.
                    den_all = small.tile([P, IB], F32, tag="denbf")
                    nc.gpsimd.partition_all_reduce(den_all[:], den_a[:], P,
                                                   ReduceOp.add)
                    rec_bc = small.tile([P, IB], F32, tag="recbc")
                    nc.vector.reciprocal(rec_bc[:], den_all[:])

                    final_ib = last_batch and ib == NIB - 1

                    def emit_norm(n_gp):
                        # normalize P and cast to fp8 (DVE first tiles,
                        # GpSimd the last -- PV consumes pairs in j order)
                        p_lo = p8_pool.tile([P, NJ, IB], F8, tag="p8")
                        for j in range(NJ - n_gp):
                            nc.vector.tensor_tensor(p_lo[:, j, :],
                                                    p_all[:, j, :],
                                                    rec_bc[:], OP.mult)
                        for j in range(NJ - n_gp, NJ):
                            nc.gpsimd.tensor_tensor(p_lo[:, j, :],
                                                    p_all[:, j, :],
                                                    rec_bc[:], OP.mult)
                        return p_lo

                    # PV+gate of the previous block runs here, hiding this
                    # block's denominator pipeline. For the very last block
                    # there is no following work to hide its softmax, so its
                    # norm is emitted first (ahead of the flush in the DVE
                    # queue) with a heavier GpSimd share.
                    if final_ib:
                        p_lo = emit_norm(8)
                        if deferred is not None:
                            flush(deferred)
                    else:
                        if deferred is not None:
                            flush(deferred)
                        p_lo = emit_norm(14 if NJ >= 16 else 0)

                    deferred = (p_lo, b, ib, v_sb)

            flush(deferred)

    nc.compile()
    return nc


def build_reps(BPC, N, D, gamma, reps=6):
    return build(BPC, N, D, gamma, reps=reps)


_CACHE = {}


def _get_nc(BPC, N, D, gamma):
    key = (BPC, N, D, float(gamma))
    if key not in _CACHE:
        _CACHE[key] = build(BPC, N, D, float(gamma))
    return _CACHE[key]


def make_in_maps(x1, x2, Wq, bq, Wk, bk, Wv, bv, W1, b1, W2, b2, W3, b3,
                 n_cores=N_CORES):
    """Host-side prep: shard over batch, transpose, cast fp8, fold weights."""
    f8 = ml_dtypes.float8_e4m3
    B, N, D = x1.shape
    DC = D // P
    Ws = (W1 + W2 + W3).astype(np.float32)
    bsum = (b1 + b2 + b3).astype(np.float32)

    def r_bias(v):  # [D] -> [128, DC] with v[c*128+p] at [p, c]
        return np.ascontiguousarray(v.reshape(DC, P).T).astype(np.float32)

    shared = {
        "wq_t": np.ascontiguousarray(WSC * Wq.T).astype(f8),
        "wk_t": np.ascontiguousarray(WSC * Wk.T).astype(f8),
        "wv_t": np.ascontiguousarray(WSC * Wv.T).astype(f8),
        "ws_t": np.ascontiguousarray(WSC * Ws.T).astype(f8),
        "b_all": np.ascontiguousarray(np.stack(
            [r_bias(WSC * bq), r_bias(WSC * bk),
             r_bias(bv), r_bias(0.5 * bsum)], axis=1)),
    }
    bpc = B // n_cores
    in_maps = []
    for c in range(n_cores):
        sl = slice(c * bpc, (c + 1) * bpc)
        in_maps.append({
            "x1t_8": np.ascontiguousarray(x1[sl].transpose(0, 2, 1)).astype(f8),
            "x2t_8": np.ascontiguousarray(x2[sl].transpose(0, 2, 1)).astype(f8),
            **shared,
        })
    return in_maps


def kernel(x1, x2, Wq, bq, Wk, bk, Wv, bv, W1, b1, W2, b2, W3, b3, gamma):
    from concourse.bass_utils import run_bass_kernel_spmd

    x1 = np.asarray(x1, dtype=np.float32)
    x2 = np.asarray(x2, dtype=np.float32)
    B, N, D = x1.shape
    bpc = B // N_CORES
    nc = _get_nc(bpc, N, D, float(np.asarray(gamma).reshape(-1)[0]))
    in_maps = make_in_maps(x1, x2, np.asarray(Wq), np.asarray(bq),
                           np.asarray(Wk), np.asarray(bk),
                           np.asarray(Wv), np.asarray(bv),
                           np.asarray(W1), np.asarray(b1),
                           np.asarray(W2), np.asarray(b2),
                           np.asarray(W3), np.asarray(b3))
    out = np.empty((B, N, D), np.float32)
    # transient axon/NRT glitches occasionally corrupt a run (non-finite
    # values); the kernel itself is deterministic, so retry on detection
    for attempt in range(3):
        res = run_bass_kernel_spmd(nc, in_maps, list(range(N_CORES)))
        for c in range(N_CORES):
            out[c * bpc:(c + 1) * bpc] = \
                res.results[c]["out"].astype(np.float32).transpose(0, 2, 1)
        if np.isfinite(out).all():
            break
    # device computed fin = (tanh((s+bs)/2) + 1) * out; the gamma/2 factor
    # and the residual are applied here
    g2 = 0.5 * float(np.asarray(gamma).reshape(-1)[0])
    out = out * g2 + x1
    return out


# revision 36
# speedup vs baseline: 1.1534x; 1.1534x over previous
"""Trainium2 Bass kernel for batched cross-attention + multiscale sigmoid gate.

Reference computation (per batch b):
    q = x1 @ Wq.T + bq ; k = x2 @ Wk.T + bk ; v = x2 @ Wv.T + bv
    attn = softmax(q @ k.T, axis=-1)              (unscaled)
    out = attn @ v
    s = out @ (W1+W2+W3).T + (b1+b2+b3)
    out = out * sigmoid(s)
    return gamma * out + x1

Strategy: pure data-parallel over batch (16 batches -> 8 cores x 2),
no collectives. Everything on-chip is kept transposed ([feature, token])
so all matmuls contract over the partition dim with zero on-device
transposes. ALL matmuls run fp8e4m3 DoubleRow (2x contraction tiles per
matmul instruction): Q/K/V projections, QK^T energy, PV, and the gate.
Weights are pre-scaled x32 on the host so their fp8 encodings stay out
of the subnormal range; every descale folds into an existing epilogue
(exp scale=1/1024, PV-drain scale=1/32, sigmoid scale=1/32) so the
rescaling is free. Softmax: fixed shift 64 (no row max), exp on ScalarE
-> bf16 P, denominator accumulated on VectorE (bf16 2x mode),
partition-all-reduced on GpSimd, reciprocal on VectorE, P normalized +
cast to fp8 split across VectorE/GpSimd. Per i-block the PV+gate of the
previous block is software-pipelined into the shadow of the current
block's softmax pipeline so TensorE never waits. Epilogues are spread
across ScalarE (Q/K drains, exp, sigmoid), GpSimd (V drain, PV drain),
and VectorE (den, norm, final gating) to keep every engine under the
fp8 TensorE roofline (~218us/core). Output is written bf16; residual
add (+x1) on the host.

Numerics (host-validated): full-output rel err 4.6e-3 vs f32 reference
(budget 2e-2). gamma ~ -0.063 scales the attention path to ~3% of the
output norm, so fp8 energy (abs energy noise ~0.5 pre-softmax) is safe.
"""

import math

import numpy as np
import ml_dtypes

import concourse.tile as tile
from concourse import mybir, bacc
from concourse.bass_isa import ReduceOp

P = 128
F32 = mybir.dt.float32
BF16 = mybir.dt.bfloat16
F8 = mybir.dt.float8e4
AF = mybir.ActivationFunctionType
OP = mybir.AluOpType
DR = mybir.MatmulPerfMode.DoubleRow

# full problem shape (hardcoded per harness contract)
B_FULL, N_FULL, D_FULL = 16, 2048, 1024
N_CORES = 8
SHIFT = 64.0
WSC = 32.0            # host-side weight scale (fp8 subnormal dodge)


def build(BPC, N, D, gamma, shift=SHIFT, reps=1):
    """Build the per-core Bass graph. BPC = batches per core."""
    DC = D // P          # feature chunks of 128
    NJ = N // P          # key tiles of 128
    PB = min(512, N)     # projection n-block
    NPB = N // PB
    IB = min(512, N)     # attention i-block (query block)
    NIB = N // IB
    KH = math.ceil(D / 512)  # V-projection k halves
    assert DC % 2 == 0 and NJ % 2 == 0

    nc = bacc.Bacc("TRN2", target_bir_lowering=False, debug=False,
                   num_devices=N_CORES)

    x1t_d = nc.declare_dram_parameter("x1t_8", [BPC, D, N], F8, isOutput=False)
    x2t_d = nc.declare_dram_parameter("x2t_8", [BPC, D, N], F8, isOutput=False)
    wq_d = nc.declare_dram_parameter("wq_t", [D, D], F8, isOutput=False)
    wk_d = nc.declare_dram_parameter("wk_t", [D, D], F8, isOutput=False)
    wv_d = nc.declare_dram_parameter("wv_t", [D, D], F8, isOutput=False)
    ws_d = nc.declare_dram_parameter("ws_t", [D, D], F8, isOutput=False)
    # all four bias vectors packed: one DMA dispatch instead of four
    ball_d = nc.declare_dram_parameter("b_all", [P, 4, DC], F32, isOutput=False)
    out_ext = nc.declare_dram_parameter("out", [BPC, D, N], BF16, isOutput=True)

    def r3(ap):  # [D, N] dram view -> [p, dc, n]
        return ap.rearrange("(c p) n -> p c n", p=P)

    def mm8(pst, lhsT3, rhs3, start, stop):
        """fp8 DoubleRow matmul over 2 contraction chunk-tiles."""
        nc.tensor.matmul(pst, lhsT=lhsT3, rhs=rhs3, start=start, stop=stop,
                         perf_mode=DR)

    with tile.TileContext(nc) as tc:
        with (
            tc.tile_pool(name="w8", bufs=1) as w8,
            tc.tile_pool(name="consts", bufs=1) as consts,
            tc.tile_pool(name="xin", bufs=4) as xin,
            tc.tile_pool(name="kv", bufs=1) as kvpool,
            tc.tile_pool(name="pall", bufs=2) as pall_pool,
            tc.tile_pool(name="p8", bufs=2) as p8_pool,
            tc.tile_pool(name="obf", bufs=2) as obf_pool,
            tc.tile_pool(name="small", bufs=1) as small,
            tc.tile_pool(name="gp", bufs=6) as gpool,
            tc.tile_pool(name="fin", bufs=3) as finpool,
            tc.tile_pool(name="psE", bufs=2, space="PSUM") as psE,
            tc.tile_pool(name="ps", bufs=4, space="PSUM") as ps,
        ):
            # constants / biases
            negshift = consts.tile([P, 1], F32)
            nc.vector.memset(negshift[:], -shift)

            # weights: fp8, resident for the whole kernel (batch-invariant).
            # Startup latency is dominated by serial DMA dispatch (~565ns
            # each), so use few, large DMAs and order them so the first Q
            # matmul group's operands land first: wq halves, x1 tile, x2
            # tile, biases. wk/wv/ws are dispatched between the first
            # projection groups.
            wq_sb = w8.tile([P, DC, D], F8, tag="wq")
            wk_sb = w8.tile([P, DC, D], F8, tag="wk")
            wv_sb = w8.tile([P, DC, D], F8, tag="wv")
            ws_sb = w8.tile([P, DC, D], F8, tag="ws")
            H = DC // 2
            nc.sync.dma_start(wq_sb[:, :H], r3(wq_d.ap())[:, :H])
            nc.sync.dma_start(wq_sb[:, H:], r3(wq_d.ap())[:, H:])
            nsl0 = slice(0, PB)
            x1t0 = xin.tile([P, DC, PB], F8, tag="xin")
            x2t0 = xin.tile([P, DC, PB], F8, tag="xin")
            nc.sync.dma_start(x1t0[:], r3(x1t_d[0])[:, :, nsl0])
            nc.sync.dma_start(x2t0[:], r3(x2t_d[0])[:, :, nsl0])
            b_all = consts.tile([P, 4, DC], F32)
            nc.sync.dma_start(b_all[:], ball_d[:])
            
            def emit_pv(p_lo, v_cur):
                # out = (P @ V')/32 + bv; drain alternates ACT/DVE (the
                # attention steady state is vector-drain-limited, not
                # matmul-limited)
                out_lo = obf_pool.tile([P, DC, IB], F8, tag="obf")
                for kc in range(DC):
                    o_ps = ps.tile([P, IB], F32, tag="ps")
                    for jp in range(NJ // 2):
                        mm8(o_ps[:],
                            v_cur[:, 2 * jp:2 * jp + 2, kc * P:(kc + 1) * P],
                            p_lo[:, 2 * jp:2 * jp + 2, :],
                            start=(jp == 0), stop=(jp == NJ // 2 - 1))
                    if kc % 2 == 0:
                        nc.scalar.activation(out_lo[:, kc, :], o_ps[:],
                                             AF.Identity,
                                             bias=b_all[:, 2, kc:kc + 1],
                                             scale=1.0 / WSC)
                    else:
                        nc.vector.tensor_scalar(
                            out_lo[:, kc, :], o_ps[:], 1.0 / WSC,
                            b_all[:, 2, kc:kc + 1], OP.mult, OP.add)
                return out_lo

            def gate_final(out_lo, b_o, ib):
                isl = slice(ib * IB, (ib + 1) * IB)
                for ec in range(DC):
                    g_ps = ps.tile([P, IB], F32, tag="ps")
                    for dc2 in range(DC // 2):
                        mm8(g_ps[:],
                            ws_sb[:, 2 * dc2:2 * dc2 + 2, ec * P:(ec + 1) * P],
                            out_lo[:, 2 * dc2:2 * dc2 + 2, :],
                            start=(dc2 == 0), stop=(dc2 == DC // 2 - 1))
                    # sigmoid(x) = 0.5*tanh(x/2) + 0.5 ; Tanh shares the
                    # ACT table with Exp/Identity, so no table reloads.
                    # bs_r is bs/2 (host-prepped); fin = (tanh+1)*out and
                    # the remaining gamma/2 factor is applied on the host.
                    g_sb = gpool.tile([P, IB], BF16, tag="g")
                    nc.scalar.activation(g_sb[:], g_ps[:], AF.Tanh,
                                         bias=b_all[:, 3, ec:ec + 1],
                                         scale=1.0 / (2.0 * WSC))
                    fin = finpool.tile([P, IB], BF16, tag="fin")
                    nc.vector.scalar_tensor_tensor(
                        fin[:], g_sb[:], 1.0,
                        out_lo[:, ec, :], OP.add, OP.mult)
                    nc.sync.dma_start(
                        out_ext[b_o, ec * P:(ec + 1) * P, isl], fin[:])

            def flush(deferred):
                # PV + gate of a pending i-block (one block, or one batch,
                # behind -- keeps TensorE fed while softmax latency drains)
                p_lo, b_o, ib, v_cur = deferred
                gate_final(emit_pv(p_lo, v_cur), b_o, ib)

            first = True
            deferred = None  # (p_lo, b, ib, v_sb) pending PV + gate
            blist = [bb for _ in range(reps) for bb in range(BPC)]
            for bi, b in enumerate(blist):
                last_batch = bi == len(blist) - 1
                # ---- phase 1: projections (all fp8 DoubleRow) ----
                qt_sb = kvpool.tile([P, DC, N], F8, tag="qt")
                kt_sb = kvpool.tile([P, DC, N], F8, tag="kt")
                v_sb = None  # allocated after the deferred flush (WAR order)

                for pb in range(NPB):
                    nsl = slice(pb * PB, (pb + 1) * PB)
                    if first and pb == 0:
                        x1t, x2t = x1t0, x2t0
                    else:
                        x1t = xin.tile([P, DC, PB], F8, tag="xin")
                        nc.sync.dma_start(x1t[:], r3(x1t_d[b])[:, :, nsl])
                        x2t = xin.tile([P, DC, PB], F8, tag="xin")
                        nc.sync.dma_start(x2t[:], r3(x2t_d[b])[:, :, nsl])

                    def q_group(ec):
                        pst = ps.tile([P, PB], F32, tag="ps")
                        for dc2 in range(DC // 2):
                            mm8(pst[:],
                                wq_sb[:, 2 * dc2:2 * dc2 + 2, ec * P:(ec + 1) * P],
                                x1t[:, 2 * dc2:2 * dc2 + 2, :],
                                start=(dc2 == 0), stop=(dc2 == DC // 2 - 1))
                        nc.scalar.activation(qt_sb[:, ec, nsl], pst[:],
                                             AF.Identity,
                                             bias=b_all[:, 0, ec:ec + 1])

                    def k_group(ec):
                        pst = ps.tile([P, PB], F32, tag="ps")
                        for dc2 in range(DC // 2):
                            mm8(pst[:],
                                wk_sb[:, 2 * dc2:2 * dc2 + 2, ec * P:(ec + 1) * P],
                                x2t[:, 2 * dc2:2 * dc2 + 2, :],
                                start=(dc2 == 0), stop=(dc2 == DC // 2 - 1))
                        nc.scalar.activation(kt_sb[:, ec, nsl], pst[:],
                                             AF.Identity,
                                             bias=b_all[:, 1, ec:ec + 1])

                    def v_group(js, kh):
                        # V chunk (resident [j, k] fp8, x32 scaled, no bias;
                        # bv added at the PV drain). Drain on DVE.
                        k0 = kh * 512
                        kw = min(512, D - k0)
                        pst = ps.tile([P, PB], F32, tag="ps")
                        for dc2 in range(DC // 2):
                            mm8(pst[:, :kw],
                                x2t[:, 2 * dc2:2 * dc2 + 2, js * P:(js + 1) * P],
                                wv_sb[:, 2 * dc2:2 * dc2 + 2, k0:k0 + kw],
                                start=(dc2 == 0), stop=(dc2 == DC // 2 - 1))
                        nc.vector.tensor_copy(
                            v_sb[:, pb * (PB // P) + js, k0:k0 + kw],
                            pst[:, :kw])

                    if first and pb == 0:
                        # blocked order with weight DMAs staged between
                        # groups so only wq + x tiles gate the start
                        for ec in range(DC):
                            q_group(ec)
                            if ec == 0:
                                for dc in range(DC):
                                    nc.sync.dma_start(wk_sb[:, dc],
                                                      r3(wk_d.ap())[:, dc])
                        for ec in range(DC):
                            k_group(ec)
                            if ec == 0:
                                for dc in range(DC):
                                    nc.sync.dma_start(wv_sb[:, dc],
                                                      r3(wv_d.ap())[:, dc])
                        v_sb = kvpool.tile([P, NJ, D], F8, tag="v")
                        for js in range(PB // P):
                            for kh in range(KH):
                                v_group(js, kh)
                        nc.sync.dma_start(ws_sb[:], r3(ws_d.ap()))
                        first = False
                    else:
                        if pb == 0:
                            # previous batch's last i-block PV+gate lands
                            # after the first K group, inside the new
                            # batch's projection stream
                            q_group(0)
                            k_group(0)
                            if deferred is not None:
                                flush(deferred)
                                deferred = None
                            v_sb = kvpool.tile([P, NJ, D], F8, tag="v")
                            for ec in range(1, DC):
                                q_group(ec)
                                k_group(ec)
                                v_group((ec - 1) // KH, (ec - 1) % KH)
                            for vg in range(DC - 1, DC):
                                v_group(vg // KH, vg % KH)
                        else:
                            # interleave Q/K/V so PSUM drains rotate over
                            # ACT/ACT/DVE (ps pool has only 4 bufs)
                            for ec in range(DC):
                                q_group(ec)
                                k_group(ec)
                                v_group(ec // KH, ec % KH)

                # ---- phase 2: attention + gate, per i-block ----
                for ib in range(NIB):
                    isl = slice(ib * IB, (ib + 1) * IB)
                    p_all = pall_pool.tile([P, NJ, IB], BF16, tag="pall")
                    den_a = small.tile([P, IB], BF16, tag="dena")

                    # pass A: energy (fp8 DR) into 2-bank PSUM tiles so a
                    # single exp covers two j-tiles (exp bias/scale are
                    # constant, so tiles can share one ACT op -- halves the
                    # ACT op count); denominator accumulated in bf16 on DVE
                    # (2x mode)
                    for jh in range(NJ // 2):
                        ps2 = psE.tile([P, 2, IB], F32, tag="ps2")
                        for h in (0, 1):
                            j = 2 * jh + h
                            for dc2 in range(DC // 2):
                                mm8(ps2[:, h],
                                    kt_sb[:, 2 * dc2:2 * dc2 + 2,
                                          j * P:(j + 1) * P],
                                    qt_sb[:, 2 * dc2:2 * dc2 + 2, isl],
                                    start=(dc2 == 0),
                                    stop=(dc2 == DC // 2 - 1))
                        nc.scalar.activation(p_all[:, 2 * jh:2 * jh + 2, :],
                                             ps2[:], AF.Exp,
                                             bias=negshift[:, 0:1],
                                             scale=1.0 / (WSC * WSC))
                        for h in (0, 1):
                            j = 2 * jh + h
                            if j == 0:
                                nc.vector.tensor_copy(den_a[:],
                                                      p_all[:, j, :])
                            else:
                                nc.vector.tensor_tensor(den_a[:], den_a[:],
                                                        p_all[:, j, :],
                                                        OP.add)

                    # partition all-reduce on GpSimd (sum + broadcast in one
                    # op), then reciprocal on DVE.
                    den_all = small.tile([P, IB], F32, tag="denbf")
                    nc.gpsimd.partition_all_reduce(den_all[:], den_a[:], P,
                                                   ReduceOp.add)
                    rec_bc = small.tile([P, IB], F32, tag="recbc")
                    nc.vector.reciprocal(rec_bc[:], den_all[:])

                    final_ib = last_batch and ib == NIB - 1

                    def emit_norm(n_gp):
                        # normalize P and cast to fp8 (DVE first tiles,
                        # GpSimd the last -- PV consumes pairs in j order)
                        p_lo = p8_pool.tile([P, NJ, IB], F8, tag="p8")
                        for j in range(NJ - n_gp):
                            nc.vector.tensor_tensor(p_lo[:, j, :],
                                                    p_all[:, j, :],
                                                    rec_bc[:], OP.mult)
                        for j in range(NJ - n_gp, NJ):
                            nc.gpsimd.tensor_tensor(p_lo[:, j, :],
                                                    p_all[:, j, :],
                                                    rec_bc[:], OP.mult)
                        return p_lo

                    # PV+gate of the previous block runs here, hiding this
                    # block's denominator pipeline. For the very last block
                    # there is no following work to hide its softmax, so its
                    # norm is emitted first (ahead of the flush in the DVE
                    # queue) with a heavier GpSimd share.
                    if final_ib:
                        p_lo = emit_norm(8)
                        if deferred is not None:
                            flush(deferred)
                    else:
                        if deferred is not None:
                            flush(deferred)
                        p_lo = emit_norm(10 if NJ >= 16 else 0)

                    deferred = (p_lo, b, ib, v_sb)

            flush(deferred)

    nc.compile()
    return nc


def build_reps(BPC, N, D, gamma, reps=6):
    return build(BPC, N, D, gamma, reps=reps)


_CACHE = {}


def _get_nc(BPC, N, D, gamma):
    key = (BPC, N, D, float(gamma))
    if key not in _CACHE:
        _CACHE[key] = build(BPC, N, D, float(gamma))
    return _CACHE[key]


def make_in_maps(x1, x2, Wq, bq, Wk, bk, Wv, bv, W1, b1, W2, b2, W3, b3,
                 n_cores=N_CORES):
    """Host-side prep: shard over batch, transpose, cast fp8, fold weights."""
    f8 = ml_dtypes.float8_e4m3
    B, N, D = x1.shape
    DC = D // P
    Ws = (W1 + W2 + W3).astype(np.float32)
    bsum = (b1 + b2 + b3).astype(np.float32)

    def r_bias(v):  # [D] -> [128, DC] with v[c*128+p] at [p, c]
        return np.ascontiguousarray(v.reshape(DC, P).T).astype(np.float32)

    shared = {
        "wq_t": np.ascontiguousarray(WSC * Wq.T).astype(f8),
        "wk_t": np.ascontiguousarray(WSC * Wk.T).astype(f8),
        "wv_t": np.ascontiguousarray(WSC * Wv.T).astype(f8),
        "ws_t": np.ascontiguousarray(WSC * Ws.T).astype(f8),
        "b_all": np.ascontiguousarray(np.stack(
            [r_bias(WSC * bq), r_bias(WSC * bk),
             r_bias(bv), r_bias(0.5 * bsum)], axis=1)),
    }
    bpc = B // n_cores
    in_maps = []
    for c in range(n_cores):
        sl = slice(c * bpc, (c + 1) * bpc)
        in_maps.append({
            "x1t_8": np.ascontiguousarray(x1[sl].transpose(0, 2, 1)).astype(f8),
            "x2t_8": np.ascontiguousarray(x2[sl].transpose(0, 2, 1)).astype(f8),
            **shared,
        })
    return in_maps


def kernel(x1, x2, Wq, bq, Wk, bk, Wv, bv, W1, b1, W2, b2, W3, b3, gamma):
    from concourse.bass_utils import run_bass_kernel_spmd

    x1 = np.asarray(x1, dtype=np.float32)
    x2 = np.asarray(x2, dtype=np.float32)
    B, N, D = x1.shape
    bpc = B // N_CORES
    nc = _get_nc(bpc, N, D, float(np.asarray(gamma).reshape(-1)[0]))
    in_maps = make_in_maps(x1, x2, np.asarray(Wq), np.asarray(bq),
                           np.asarray(Wk), np.asarray(bk),
                           np.asarray(Wv), np.asarray(bv),
                           np.asarray(W1), np.asarray(b1),
                           np.asarray(W2), np.asarray(b2),
                           np.asarray(W3), np.asarray(b3))
    out = np.empty((B, N, D), np.float32)
    # transient axon/NRT glitches occasionally corrupt a run (non-finite
    # values); the kernel itself is deterministic, so retry on detection
    for attempt in range(3):
        res = run_bass_kernel_spmd(nc, in_maps, list(range(N_CORES)))
        for c in range(N_CORES):
            out[c * bpc:(c + 1) * bpc] = \
                res.results[c]["out"].astype(np.float32).transpose(0, 2, 1)
        if np.isfinite(out).all():
            break
    # device computed fin = (tanh((s+bs)/2) + 1) * out; the gamma/2 factor
    # and the residual are applied here
    g2 = 0.5 * float(np.asarray(gamma).reshape(-1)[0])
    out = out * g2 + x1
    return out


# revision 37
# speedup vs baseline: 1.1808x; 1.0238x over previous
"""Trainium2 Bass kernel for batched cross-attention + multiscale sigmoid gate.

Reference computation (per batch b):
    q = x1 @ Wq.T + bq ; k = x2 @ Wk.T + bk ; v = x2 @ Wv.T + bv
    attn = softmax(q @ k.T, axis=-1)              (unscaled)
    out = attn @ v
    s = out @ (W1+W2+W3).T + (b1+b2+b3)
    out = out * sigmoid(s)
    return gamma * out + x1

Strategy: pure data-parallel over batch (16 batches -> 8 cores x 2),
no collectives. Everything on-chip is kept transposed ([feature, token])
so all matmuls contract over the partition dim with zero on-device
transposes. ALL matmuls run fp8e4m3 DoubleRow (2x contraction tiles per
matmul instruction): Q/K/V projections, QK^T energy, PV, and the gate.
Weights are pre-scaled x32 on the host so their fp8 encodings stay out
of the subnormal range; every descale folds into an existing epilogue
(exp scale=1/1024, PV-drain scale=1/32, sigmoid scale=1/32) so the
rescaling is free. Softmax: fixed shift 64 (no row max), exp on ScalarE
-> bf16 P, denominator accumulated on VectorE (bf16 2x mode),
partition-all-reduced on GpSimd, reciprocal on VectorE, P normalized +
cast to fp8 split across VectorE/GpSimd. Per i-block the PV+gate of the
previous block is software-pipelined into the shadow of the current
block's softmax pipeline so TensorE never waits. Epilogues are spread
across ScalarE (Q/K drains, exp, sigmoid), GpSimd (V drain, PV drain),
and VectorE (den, norm, final gating) to keep every engine under the
fp8 TensorE roofline (~218us/core). Output is written bf16; residual
add (+x1) on the host.

Numerics (host-validated): full-output rel err 4.6e-3 vs f32 reference
(budget 2e-2). gamma ~ -0.063 scales the attention path to ~3% of the
output norm, so fp8 energy (abs energy noise ~0.5 pre-softmax) is safe.
"""

import math

import numpy as np
import ml_dtypes

import concourse.tile as tile
from concourse import mybir, bacc
from concourse.bass_isa import ReduceOp

P = 128
F32 = mybir.dt.float32
BF16 = mybir.dt.bfloat16
F8 = mybir.dt.float8e4
AF = mybir.ActivationFunctionType
OP = mybir.AluOpType
DR = mybir.MatmulPerfMode.DoubleRow

# full problem shape (hardcoded per harness contract)
B_FULL, N_FULL, D_FULL = 16, 2048, 1024
N_CORES = 8
SHIFT = 64.0
WSC = 32.0            # host-side weight scale (fp8 subnormal dodge)


def build(BPC, N, D, gamma, shift=SHIFT, reps=1):
    """Build the per-core Bass graph. BPC = batches per core."""
    DC = D // P          # feature chunks of 128
    NJ = N // P          # key tiles of 128
    PB = min(512, N)     # projection n-block
    NPB = N // PB
    IB = min(512, N)     # attention i-block (query block)
    NIB = N // IB
    KH = math.ceil(D / 512)  # V-projection k halves
    assert DC % 2 == 0 and NJ % 2 == 0

    nc = bacc.Bacc("TRN2", target_bir_lowering=False, debug=False,
                   num_devices=N_CORES)

    x1t_d = nc.declare_dram_parameter("x1t_8", [BPC, D, N], F8, isOutput=False)
    x2t_d = nc.declare_dram_parameter("x2t_8", [BPC, D, N], F8, isOutput=False)
    wq_d = nc.declare_dram_parameter("wq_t", [D, D], F8, isOutput=False)
    wk_d = nc.declare_dram_parameter("wk_t", [D, D], F8, isOutput=False)
    wv_d = nc.declare_dram_parameter("wv_t", [D, D], F8, isOutput=False)
    ws_d = nc.declare_dram_parameter("ws_t", [D, D], F8, isOutput=False)
    # all four bias vectors packed: one DMA dispatch instead of four
    ball_d = nc.declare_dram_parameter("b_all", [P, 4, DC], F32, isOutput=False)
    out_ext = nc.declare_dram_parameter("out", [BPC, D, N], BF16, isOutput=True)

    def r3(ap):  # [D, N] dram view -> [p, dc, n]
        return ap.rearrange("(c p) n -> p c n", p=P)

    def mm8(pst, lhsT3, rhs3, start, stop):
        """fp8 DoubleRow matmul over 2 contraction chunk-tiles."""
        nc.tensor.matmul(pst, lhsT=lhsT3, rhs=rhs3, start=start, stop=stop,
                         perf_mode=DR)

    with tile.TileContext(nc) as tc:
        with (
            tc.tile_pool(name="w8", bufs=1) as w8,
            tc.tile_pool(name="consts", bufs=1) as consts,
            tc.tile_pool(name="xin", bufs=4) as xin,
            tc.tile_pool(name="kv", bufs=1) as kvpool,
            tc.tile_pool(name="pall", bufs=2) as pall_pool,
            tc.tile_pool(name="p8", bufs=2) as p8_pool,
            tc.tile_pool(name="obf", bufs=2) as obf_pool,
            tc.tile_pool(name="small", bufs=1) as small,
            tc.tile_pool(name="gp", bufs=6) as gpool,
            tc.tile_pool(name="fin", bufs=3) as finpool,
            tc.tile_pool(name="psE", bufs=2, space="PSUM") as psE,
            tc.tile_pool(name="ps", bufs=4, space="PSUM") as ps,
        ):
            # constants / biases
            negshift = consts.tile([P, 1], F32)
            nc.vector.memset(negshift[:], -shift)

            # weights: fp8, resident for the whole kernel (batch-invariant).
            # Startup latency is dominated by serial DMA dispatch (~565ns
            # each), so use few, large DMAs and order them so the first Q
            # matmul group's operands land first: wq halves, x1 tile, x2
            # tile, biases. wk/wv/ws are dispatched between the first
            # projection groups.
            wq_sb = w8.tile([P, DC, D], F8, tag="wq")
            wk_sb = w8.tile([P, DC, D], F8, tag="wk")
            wv_sb = w8.tile([P, DC, D], F8, tag="wv")
            ws_sb = w8.tile([P, DC, D], F8, tag="ws")
            H = DC // 2
            nc.sync.dma_start(wq_sb[:, :H], r3(wq_d.ap())[:, :H])
            nc.sync.dma_start(wq_sb[:, H:], r3(wq_d.ap())[:, H:])
            nsl0 = slice(0, PB)
            x1t0 = xin.tile([P, DC, PB], F8, tag="xin")
            x2t0 = xin.tile([P, DC, PB], F8, tag="xin")
            nc.sync.dma_start(x1t0[:], r3(x1t_d[0])[:, :, nsl0])
            nc.sync.dma_start(x2t0[:], r3(x2t_d[0])[:, :, nsl0])
            b_all = consts.tile([P, 4, DC], F32)
            nc.sync.dma_start(b_all[:], ball_d[:])
            
            def emit_pv(p_lo, v_cur):
                # out = (P @ V')/32 + bv; drain alternates ACT/DVE (the
                # attention steady state is vector-drain-limited, not
                # matmul-limited)
                out_lo = obf_pool.tile([P, DC, IB], F8, tag="obf")
                for kc in range(DC):
                    o_ps = ps.tile([P, IB], F32, tag="ps")
                    for jp in range(NJ // 2):
                        mm8(o_ps[:],
                            v_cur[:, 2 * jp:2 * jp + 2, kc * P:(kc + 1) * P],
                            p_lo[:, 2 * jp:2 * jp + 2, :],
                            start=(jp == 0), stop=(jp == NJ // 2 - 1))
                    if kc % 2 == 0:
                        nc.scalar.activation(out_lo[:, kc, :], o_ps[:],
                                             AF.Identity,
                                             bias=b_all[:, 2, kc:kc + 1],
                                             scale=1.0 / WSC)
                    else:
                        nc.vector.tensor_scalar(
                            out_lo[:, kc, :], o_ps[:], 1.0 / WSC,
                            b_all[:, 2, kc:kc + 1], OP.mult, OP.add)
                return out_lo

            def gate_final(out_lo, b_o, ib):
                isl = slice(ib * IB, (ib + 1) * IB)
                for ec in range(DC):
                    g_ps = ps.tile([P, IB], F32, tag="ps")
                    for dc2 in range(DC // 2):
                        mm8(g_ps[:],
                            ws_sb[:, 2 * dc2:2 * dc2 + 2, ec * P:(ec + 1) * P],
                            out_lo[:, 2 * dc2:2 * dc2 + 2, :],
                            start=(dc2 == 0), stop=(dc2 == DC // 2 - 1))
                    # sigmoid(x) = 0.5*tanh(x/2) + 0.5 ; Tanh shares the
                    # ACT table with Exp/Identity, so no table reloads.
                    # bs_r is bs/2 (host-prepped); fin = (tanh+1)*out and
                    # the remaining gamma/2 factor is applied on the host.
                    g_sb = gpool.tile([P, IB], BF16, tag="g")
                    nc.scalar.activation(g_sb[:], g_ps[:], AF.Tanh,
                                         bias=b_all[:, 3, ec:ec + 1],
                                         scale=1.0 / (2.0 * WSC))
                    fin = finpool.tile([P, IB], BF16, tag="fin")
                    nc.vector.scalar_tensor_tensor(
                        fin[:], g_sb[:], 1.0,
                        out_lo[:, ec, :], OP.add, OP.mult)
                    nc.sync.dma_start(
                        out_ext[b_o, ec * P:(ec + 1) * P, isl], fin[:])

            def flush(deferred):
                # PV + gate of a pending i-block (one block, or one batch,
                # behind -- keeps TensorE fed while softmax latency drains)
                p_lo, b_o, ib, v_cur = deferred
                gate_final(emit_pv(p_lo, v_cur), b_o, ib)

            first = True
            deferred = None  # (p_lo, b, ib, v_sb) pending PV + gate
            blist = [bb for _ in range(reps) for bb in range(BPC)]
            for bi, b in enumerate(blist):
                last_batch = bi == len(blist) - 1
                # ---- phase 1: projections (all fp8 DoubleRow) ----
                qt_sb = kvpool.tile([P, DC, N], F8, tag="qt")
                kt_sb = kvpool.tile([P, DC, N], F8, tag="kt")
                v_sb = None  # allocated after the deferred flush (WAR order)

                for pb in range(NPB):
                    nsl = slice(pb * PB, (pb + 1) * PB)
                    if first and pb == 0:
                        x1t, x2t = x1t0, x2t0
                    else:
                        x1t = xin.tile([P, DC, PB], F8, tag="xin")
                        nc.sync.dma_start(x1t[:], r3(x1t_d[b])[:, :, nsl])
                        x2t = xin.tile([P, DC, PB], F8, tag="xin")
                        nc.sync.dma_start(x2t[:], r3(x2t_d[b])[:, :, nsl])

                    def q_group(ec):
                        pst = ps.tile([P, PB], F32, tag="ps")
                        for dc2 in range(DC // 2):
                            mm8(pst[:],
                                wq_sb[:, 2 * dc2:2 * dc2 + 2, ec * P:(ec + 1) * P],
                                x1t[:, 2 * dc2:2 * dc2 + 2, :],
                                start=(dc2 == 0), stop=(dc2 == DC // 2 - 1))
                        nc.scalar.activation(qt_sb[:, ec, nsl], pst[:],
                                             AF.Identity,
                                             bias=b_all[:, 0, ec:ec + 1])

                    def k_group(ec):
                        pst = ps.tile([P, PB], F32, tag="ps")
                        for dc2 in range(DC // 2):
                            mm8(pst[:],
                                wk_sb[:, 2 * dc2:2 * dc2 + 2, ec * P:(ec + 1) * P],
                                x2t[:, 2 * dc2:2 * dc2 + 2, :],
                                start=(dc2 == 0), stop=(dc2 == DC // 2 - 1))
                        nc.scalar.activation(kt_sb[:, ec, nsl], pst[:],
                                             AF.Identity,
                                             bias=b_all[:, 1, ec:ec + 1])

                    def v_group(js, kh):
                        # V chunk (resident [j, k] fp8, x32 scaled, no bias;
                        # bv added at the PV drain). Drain on DVE.
                        k0 = kh * 512
                        kw = min(512, D - k0)
                        pst = ps.tile([P, PB], F32, tag="ps")
                        for dc2 in range(DC // 2):
                            mm8(pst[:, :kw],
                                x2t[:, 2 * dc2:2 * dc2 + 2, js * P:(js + 1) * P],
                                wv_sb[:, 2 * dc2:2 * dc2 + 2, k0:k0 + kw],
                                start=(dc2 == 0), stop=(dc2 == DC // 2 - 1))
                        nc.vector.tensor_copy(
                            v_sb[:, pb * (PB // P) + js, k0:k0 + kw],
                            pst[:, :kw])

                    if first and pb == 0:
                        # blocked order with weight DMAs staged between
                        # groups so only wq + x tiles gate the start
                        for ec in range(DC):
                            q_group(ec)
                            if ec == 0:
                                for dc in range(DC):
                                    nc.sync.dma_start(wk_sb[:, dc],
                                                      r3(wk_d.ap())[:, dc])
                        for ec in range(DC):
                            k_group(ec)
                            if ec == 0:
                                for dc in range(DC):
                                    nc.sync.dma_start(wv_sb[:, dc],
                                                      r3(wv_d.ap())[:, dc])
                        v_sb = kvpool.tile([P, NJ, D], F8, tag="v")
                        for js in range(PB // P):
                            for kh in range(KH):
                                v_group(js, kh)
                        nc.sync.dma_start(ws_sb[:], r3(ws_d.ap()))
                        first = False
                    else:
                        if pb == 0:
                            # previous batch's last i-block PV+gate lands
                            # after the first K group, inside the new
                            # batch's projection stream
                            q_group(0)
                            k_group(0)
                            if deferred is not None:
                                flush(deferred)
                                deferred = None
                            v_sb = kvpool.tile([P, NJ, D], F8, tag="v")
                            for ec in range(1, DC):
                                q_group(ec)
                                k_group(ec)
                                v_group((ec - 1) // KH, (ec - 1) % KH)
                            for vg in range(DC - 1, DC):
                                v_group(vg // KH, vg % KH)
                        else:
                            # interleave Q/K/V so PSUM drains rotate over
                            # ACT/ACT/DVE (ps pool has only 4 bufs)
                            for ec in range(DC):
                                q_group(ec)
                                k_group(ec)
                                v_group(ec // KH, ec % KH)

                # ---- phase 2: attention + gate, per i-block ----
                for ib in range(NIB):
                    isl = slice(ib * IB, (ib + 1) * IB)
                    p_all = pall_pool.tile([P, NJ, IB], BF16, tag="pall")
                    den_a = small.tile([P, IB], BF16, tag="dena")

                    # pass A: energy (fp8 DR) into 2-bank PSUM tiles so a
                    # single exp covers two j-tiles (exp bias/scale are
                    # constant, so tiles can share one ACT op -- halves the
                    # ACT op count); denominator accumulated in bf16 on DVE
                    # (2x mode)
                    for jh in range(NJ // 2):
                        ps2 = psE.tile([P, 2, IB], F32, tag="ps2")
                        for h in (0, 1):
                            j = 2 * jh + h
                            for dc2 in range(DC // 2):
                                mm8(ps2[:, h],
                                    kt_sb[:, 2 * dc2:2 * dc2 + 2,
                                          j * P:(j + 1) * P],
                                    qt_sb[:, 2 * dc2:2 * dc2 + 2, isl],
                                    start=(dc2 == 0),
                                    stop=(dc2 == DC // 2 - 1))
                        nc.scalar.activation(p_all[:, 2 * jh:2 * jh + 2, :],
                                             ps2[:], AF.Exp,
                                             bias=negshift[:, 0:1],
                                             scale=1.0 / (WSC * WSC))
                        for h in (0, 1):
                            j = 2 * jh + h
                            if j == 0:
                                nc.vector.tensor_copy(den_a[:],
                                                      p_all[:, j, :])
                            else:
                                nc.vector.tensor_tensor(den_a[:], den_a[:],
                                                        p_all[:, j, :],
                                                        OP.add)

                    # partition all-reduce on GpSimd (sum + broadcast in one
                    # op), then reciprocal on DVE.
                    den_all = small.tile([P, IB], F32, tag="denbf")
                    nc.gpsimd.partition_all_reduce(den_all[:], den_a[:], P,
                                                   ReduceOp.add)
                    rec_bc = small.tile([P, IB], F32, tag="recbc")
                    nc.vector.reciprocal(rec_bc[:], den_all[:])

                    final_ib = last_batch and ib == NIB - 1

                    def emit_norm(n_gp):
                        # normalize P and cast to fp8 (DVE first tiles,
                        # GpSimd the last -- PV consumes pairs in j order)
                        p_lo = p8_pool.tile([P, NJ, IB], F8, tag="p8")
                        for j in range(NJ - n_gp):
                            nc.vector.tensor_tensor(p_lo[:, j, :],
                                                    p_all[:, j, :],
                                                    rec_bc[:], OP.mult)
                        for j in range(NJ - n_gp, NJ):
                            nc.gpsimd.tensor_tensor(p_lo[:, j, :],
                                                    p_all[:, j, :],
                                                    rec_bc[:], OP.mult)
                        return p_lo

                    # PV+gate of the previous block runs here, hiding this
                    # block's denominator pipeline. For the very last block
                    # there is no following work to hide its softmax, so its
                    # norm is emitted first (ahead of the flush in the DVE
                    # queue) with a heavier GpSimd share.
                    if final_ib:
                        p_lo = emit_norm(8)
                        if deferred is not None:
                            flush(deferred)
                    else:
                        if deferred is not None:
                            flush(deferred)
                        # ib0 norm races PV(ib0) at the ib0->ib1 boundary
                        # (no flush spaces them), so give Pool less of it
                        n_gp = (6 if ib == 0 else 10) if NJ >= 16 else 0
                        p_lo = emit_norm(n_gp)

                    deferred = (p_lo, b, ib, v_sb)

            flush(deferred)

    nc.compile()
    return nc


def build_reps(BPC, N, D, gamma, reps=6):
    return build(BPC, N, D, gamma, reps=reps)


_CACHE = {}


def _get_nc(BPC, N, D, gamma):
    key = (BPC, N, D, float(gamma))
    if key not in _CACHE:
        _CACHE[key] = build(BPC, N, D, float(gamma))
    return _CACHE[key]


def make_in_maps(x1, x2, Wq, bq, Wk, bk, Wv, bv, W1, b1, W2, b2, W3, b3,
                 n_cores=N_CORES):
    """Host-side prep: shard over batch, transpose, cast fp8, fold weights."""
    f8 = ml_dtypes.float8_e4m3
    B, N, D = x1.shape
    DC = D // P
    Ws = (W1 + W2 + W3).astype(np.float32)
    bsum = (b1 + b2 + b3).astype(np.float32)

    def r_bias(v):  # [D] -> [128, DC] with v[c*128+p] at [p, c]
        return np.ascontiguousarray(v.reshape(DC, P).T).astype(np.float32)

    shared = {
        "wq_t": np.ascontiguousarray(WSC * Wq.T).astype(f8),
        "wk_t": np.ascontiguousarray(WSC * Wk.T).astype(f8),
        "wv_t": np.ascontiguousarray(WSC * Wv.T).astype(f8),
        "ws_t": np.ascontiguousarray(WSC * Ws.T).astype(f8),
        "b_all": np.ascontiguousarray(np.stack(
            [r_bias(WSC * bq), r_bias(WSC * bk),
             r_bias(bv), r_bias(0.5 * bsum)], axis=1)),
    }
    bpc = B // n_cores
    in_maps = []
    for c in range(n_cores):
        sl = slice(c * bpc, (c + 1) * bpc)
        in_maps.append({
            "x1t_8": np.ascontiguousarray(x1[sl].transpose(0, 2, 1)).astype(f8),
            "x2t_8": np.ascontiguousarray(x2[sl].transpose(0, 2, 1)).astype(f8),
            **shared,
        })
    return in_maps


def kernel(x1, x2, Wq, bq, Wk, bk, Wv, bv, W1, b1, W2, b2, W3, b3, gamma):
    from concourse.bass_utils import run_bass_kernel_spmd

    x1 = np.asarray(x1, dtype=np.float32)
    x2 = np.asarray(x2, dtype=np.float32)
    B, N, D = x1.shape
    bpc = B // N_CORES
    nc = _get_nc(bpc, N, D, float(np.asarray(gamma).reshape(-1)[0]))
    in_maps = make_in_maps(x1, x2, np.asarray(Wq), np.asarray(bq),
                           np.asarray(Wk), np.asarray(bk),
                           np.asarray(Wv), np.asarray(bv),
                           np.asarray(W1), np.asarray(b1),
                           np.asarray(W2), np.asarray(b2),
                           np.asarray(W3), np.asarray(b3))
    out = np.empty((B, N, D), np.float32)
    # transient axon/NRT glitches occasionally corrupt a run (non-finite
    # values); the kernel itself is deterministic, so retry on detection
    for attempt in range(3):
        res = run_bass_kernel_spmd(nc, in_maps, list(range(N_CORES)))
        for c in range(N_CORES):
            out[c * bpc:(c + 1) * bpc] = \
                res.results[c]["out"].astype(np.float32).transpose(0, 2, 1)
        if np.isfinite(out).all():
            break
    # device computed fin = (tanh((s+bs)/2) + 1) * out; the gamma/2 factor
    # and the residual are applied here
    g2 = 0.5 * float(np.asarray(gamma).reshape(-1)[0])
    out = out * g2 + x1
    return out


# revision 41
# speedup vs baseline: 1.1838x; 1.0026x over previous
"""Trainium2 Bass kernel for batched cross-attention + multiscale sigmoid gate.

Reference computation (per batch b):
    q = x1 @ Wq.T + bq ; k = x2 @ Wk.T + bk ; v = x2 @ Wv.T + bv
    attn = softmax(q @ k.T, axis=-1)              (unscaled)
    out = attn @ v
    s = out @ (W1+W2+W3).T + (b1+b2+b3)
    out = out * sigmoid(s)
    return gamma * out + x1

Strategy: pure data-parallel over batch (16 batches -> 8 cores x 2),
no collectives. Everything on-chip is kept transposed ([feature, token])
so all matmuls contract over the partition dim with zero on-device
transposes. ALL matmuls run fp8e4m3 DoubleRow (2x contraction tiles per
matmul instruction): Q/K/V projections, QK^T energy, PV, and the gate.
Weights are pre-scaled x32 on the host so their fp8 encodings stay out
of the subnormal range; every descale folds into an existing epilogue
(exp scale=1/1024, PV-drain scale=1/32, sigmoid scale=1/32) so the
rescaling is free. Softmax: fixed shift 64 (no row max), exp on ScalarE
-> bf16 P, denominator accumulated on VectorE (bf16 2x mode),
partition-all-reduced on GpSimd, reciprocal on VectorE, P normalized +
cast to fp8 split across VectorE/GpSimd. Per i-block the PV+gate of the
previous block is software-pipelined into the shadow of the current
block's softmax pipeline so TensorE never waits. Epilogues are spread
across ScalarE (Q/K drains, exp, sigmoid), GpSimd (V drain, PV drain),
and VectorE (den, norm, final gating) to keep every engine under the
fp8 TensorE roofline (~218us/core). Output is written bf16; residual
add (+x1) on the host.

Numerics (host-validated): full-output rel err 4.6e-3 vs f32 reference
(budget 2e-2). gamma ~ -0.063 scales the attention path to ~3% of the
output norm, so fp8 energy (abs energy noise ~0.5 pre-softmax) is safe.
"""

import math

import numpy as np
import ml_dtypes

import concourse.tile as tile
from concourse import mybir, bacc
from concourse.bass_isa import ReduceOp

P = 128
F32 = mybir.dt.float32
BF16 = mybir.dt.bfloat16
F8 = mybir.dt.float8e4
AF = mybir.ActivationFunctionType
OP = mybir.AluOpType
DR = mybir.MatmulPerfMode.DoubleRow

# full problem shape (hardcoded per harness contract)
B_FULL, N_FULL, D_FULL = 16, 2048, 1024
N_CORES = 8
SHIFT = 64.0
WSC = 32.0            # host-side weight scale (fp8 subnormal dodge)


def build(BPC, N, D, gamma, shift=SHIFT, reps=1):
    """Build the per-core Bass graph. BPC = batches per core."""
    DC = D // P          # feature chunks of 128
    NJ = N // P          # key tiles of 128
    PB = min(512, N)     # projection n-block
    NPB = N // PB
    IB = min(512, N)     # attention i-block (query block)
    NIB = N // IB
    KH = math.ceil(D / 512)  # V-projection k halves
    assert DC % 2 == 0 and NJ % 2 == 0

    nc = bacc.Bacc("TRN2", target_bir_lowering=False, debug=False,
                   num_devices=N_CORES)

    x1t_d = nc.declare_dram_parameter("x1t_8", [BPC, D, N], F8, isOutput=False)
    x2t_d = nc.declare_dram_parameter("x2t_8", [BPC, D, N], F8, isOutput=False)
    wq_d = nc.declare_dram_parameter("wq_t", [D, D], F8, isOutput=False)
    wk_d = nc.declare_dram_parameter("wk_t", [D, D], F8, isOutput=False)
    wv_d = nc.declare_dram_parameter("wv_t", [D, D], F8, isOutput=False)
    ws_d = nc.declare_dram_parameter("ws_t", [D, D], F8, isOutput=False)
    # all four bias vectors packed: one DMA dispatch instead of four
    ball_d = nc.declare_dram_parameter("b_all", [P, 4, DC], F32, isOutput=False)
    out_ext = nc.declare_dram_parameter("out", [BPC, D, N], BF16, isOutput=True)

    def r3(ap):  # [D, N] dram view -> [p, dc, n]
        return ap.rearrange("(c p) n -> p c n", p=P)

    def mm8(pst, lhsT3, rhs3, start, stop):
        """fp8 DoubleRow matmul over 2 contraction chunk-tiles."""
        nc.tensor.matmul(pst, lhsT=lhsT3, rhs=rhs3, start=start, stop=stop,
                         perf_mode=DR)

    with tile.TileContext(nc) as tc:
        with (
            tc.tile_pool(name="w8", bufs=1) as w8,
            tc.tile_pool(name="consts", bufs=1) as consts,
            tc.tile_pool(name="xin", bufs=4) as xin,
            tc.tile_pool(name="kv", bufs=1) as kvpool,
            tc.tile_pool(name="pall", bufs=2) as pall_pool,
            tc.tile_pool(name="p8", bufs=2) as p8_pool,
            tc.tile_pool(name="obf", bufs=2) as obf_pool,
            tc.tile_pool(name="small", bufs=1) as small,
            tc.tile_pool(name="gp", bufs=6) as gpool,
            tc.tile_pool(name="fin", bufs=3) as finpool,
            tc.tile_pool(name="psE", bufs=2, space="PSUM") as psE,
            tc.tile_pool(name="ps", bufs=4, space="PSUM") as ps,
        ):
            # constants / biases
            negshift = consts.tile([P, 1], F32)
            nc.vector.memset(negshift[:], -shift)

            # weights: fp8, resident for the whole kernel (batch-invariant).
            # Startup latency is dominated by serial DMA dispatch (~565ns
            # each), so use few, large DMAs and order them so the first Q
            # matmul group's operands land first: wq halves, x1 tile, x2
            # tile, biases. wk/wv/ws are dispatched between the first
            # projection groups.
            wq_sb = w8.tile([P, DC, D], F8, tag="wq")
            wk_sb = w8.tile([P, DC, D], F8, tag="wk")
            wv_sb = w8.tile([P, DC, D], F8, tag="wv")
            ws_sb = w8.tile([P, DC, D], F8, tag="ws")
            H = DC // 2
            nsl0 = slice(0, PB)
            x1t0 = xin.tile([P, DC, PB], F8, tag="xin")
            x2t0 = xin.tile([P, DC, PB], F8, tag="xin")
            nc.sync.dma_start(wq_sb[:, :H], r3(wq_d.ap())[:, :H])
            nc.sync.dma_start(x1t0[:, :H], r3(x1t_d[0])[:, :H, nsl0])
            nc.sync.dma_start(wq_sb[:, H:], r3(wq_d.ap())[:, H:])
            nc.sync.dma_start(x1t0[:, H:], r3(x1t_d[0])[:, H:, nsl0])
            b_all = consts.tile([P, 4, DC], F32)
            nc.sync.dma_start(b_all[:], ball_d[:])
            nc.sync.dma_start(x2t0[:, :H], r3(x2t_d[0])[:, :H, nsl0])
            nc.sync.dma_start(x2t0[:, H:], r3(x2t_d[0])[:, H:, nsl0])
            
            def emit_pv(p_lo, v_cur):
                # out = (P @ V')/32 + bv; drain alternates ACT/DVE (the
                # attention steady state is vector-drain-limited, not
                # matmul-limited)
                out_lo = obf_pool.tile([P, DC, IB], F8, tag="obf")
                for kc in range(DC):
                    o_ps = ps.tile([P, IB], F32, tag="ps")
                    for jp in range(NJ // 2):
                        mm8(o_ps[:],
                            v_cur[:, 2 * jp:2 * jp + 2, kc * P:(kc + 1) * P],
                            p_lo[:, 2 * jp:2 * jp + 2, :],
                            start=(jp == 0), stop=(jp == NJ // 2 - 1))
                    if kc % 2 == 0:
                        nc.scalar.activation(out_lo[:, kc, :], o_ps[:],
                                             AF.Identity,
                                             bias=b_all[:, 2, kc:kc + 1],
                                             scale=1.0 / WSC)
                    else:
                        nc.vector.tensor_scalar(
                            out_lo[:, kc, :], o_ps[:], 1.0 / WSC,
                            b_all[:, 2, kc:kc + 1], OP.mult, OP.add)
                return out_lo

            def gate_final(out_lo, b_o, ib):
                isl = slice(ib * IB, (ib + 1) * IB)
                for ec in range(DC):
                    g_ps = ps.tile([P, IB], F32, tag="ps")
                    for dc2 in range(DC // 2):
                        mm8(g_ps[:],
                            ws_sb[:, 2 * dc2:2 * dc2 + 2, ec * P:(ec + 1) * P],
                            out_lo[:, 2 * dc2:2 * dc2 + 2, :],
                            start=(dc2 == 0), stop=(dc2 == DC // 2 - 1))
                    # sigmoid(x) = 0.5*tanh(x/2) + 0.5 ; Tanh shares the
                    # ACT table with Exp/Identity, so no table reloads.
                    # bs_r is bs/2 (host-prepped); fin = (tanh+1)*out and
                    # the remaining gamma/2 factor is applied on the host.
                    g_sb = gpool.tile([P, IB], BF16, tag="g")
                    nc.scalar.activation(g_sb[:], g_ps[:], AF.Tanh,
                                         bias=b_all[:, 3, ec:ec + 1],
                                         scale=1.0 / (2.0 * WSC))
                    fin = finpool.tile([P, IB], BF16, tag="fin")
                    nc.vector.scalar_tensor_tensor(
                        fin[:], g_sb[:], 1.0,
                        out_lo[:, ec, :], OP.add, OP.mult)
                    nc.sync.dma_start(
                        out_ext[b_o, ec * P:(ec + 1) * P, isl], fin[:])

            def flush(deferred):
                # PV + gate of a pending i-block (one block, or one batch,
                # behind -- keeps TensorE fed while softmax latency drains)
                p_lo, b_o, ib, v_cur = deferred
                gate_final(emit_pv(p_lo, v_cur), b_o, ib)

            first = True
            deferred = None  # (p_lo, b, ib, v_sb) pending PV + gate
            pre_x = None     # next batch's first x tiles, prefetched
            blist = [bb for _ in range(reps) for bb in range(BPC)]
            for bi, b in enumerate(blist):
                last_batch = bi == len(blist) - 1
                # ---- phase 1: projections (all fp8 DoubleRow) ----
                qt_sb = kvpool.tile([P, DC, N], F8, tag="qt")
                kt_sb = kvpool.tile([P, DC, N], F8, tag="kt")
                v_sb = None  # allocated after the deferred flush (WAR order)

                for pb in range(NPB):
                    nsl = slice(pb * PB, (pb + 1) * PB)
                    if first and pb == 0:
                        x1t, x2t = x1t0, x2t0
                    elif pb == 0 and pre_x is not None:
                        x1t, x2t = pre_x  # prefetched in prev batch's tail
                        pre_x = None
                    else:
                        x1t = xin.tile([P, DC, PB], F8, tag="xin")
                        nc.sync.dma_start(x1t[:], r3(x1t_d[b])[:, :, nsl])
                        x2t = xin.tile([P, DC, PB], F8, tag="xin")
                        nc.sync.dma_start(x2t[:], r3(x2t_d[b])[:, :, nsl])

                    def q_group(ec):
                        pst = ps.tile([P, PB], F32, tag="ps")
                        for dc2 in range(DC // 2):
                            mm8(pst[:],
                                wq_sb[:, 2 * dc2:2 * dc2 + 2, ec * P:(ec + 1) * P],
                                x1t[:, 2 * dc2:2 * dc2 + 2, :],
                                start=(dc2 == 0), stop=(dc2 == DC // 2 - 1))
                        nc.scalar.activation(qt_sb[:, ec, nsl], pst[:],
                                             AF.Identity,
                                             bias=b_all[:, 0, ec:ec + 1])

                    def k_group(ec):
                        pst = ps.tile([P, PB], F32, tag="ps")
                        for dc2 in range(DC // 2):
                            mm8(pst[:],
                                wk_sb[:, 2 * dc2:2 * dc2 + 2, ec * P:(ec + 1) * P],
                                x2t[:, 2 * dc2:2 * dc2 + 2, :],
                                start=(dc2 == 0), stop=(dc2 == DC // 2 - 1))
                        nc.scalar.activation(kt_sb[:, ec, nsl], pst[:],
                                             AF.Identity,
                                             bias=b_all[:, 1, ec:ec + 1])

                    def v_group(js, kh):
                        # V chunk (resident [j, k] fp8, x32 scaled, no bias;
                        # bv added at the PV drain). Drain on DVE.
                        k0 = kh * 512
                        kw = min(512, D - k0)
                        pst = ps.tile([P, PB], F32, tag="ps")
                        for dc2 in range(DC // 2):
                            mm8(pst[:, :kw],
                                x2t[:, 2 * dc2:2 * dc2 + 2, js * P:(js + 1) * P],
                                wv_sb[:, 2 * dc2:2 * dc2 + 2, k0:k0 + kw],
                                start=(dc2 == 0), stop=(dc2 == DC // 2 - 1))
                        nc.vector.tensor_copy(
                            v_sb[:, pb * (PB // P) + js, k0:k0 + kw],
                            pst[:, :kw])

                    if first and pb == 0:
                        # blocked order with weight DMAs staged between
                        # groups so only wq + x tiles gate the start
                        for ec in range(DC):
                            q_group(ec)
                            if ec == 0:
                                for dc in range(DC):
                                    nc.sync.dma_start(wk_sb[:, dc],
                                                      r3(wk_d.ap())[:, dc])
                        for ec in range(DC):
                            k_group(ec)
                            if ec == 0:
                                for dc in range(DC):
                                    nc.sync.dma_start(wv_sb[:, dc],
                                                      r3(wv_d.ap())[:, dc])
                        v_sb = kvpool.tile([P, NJ, D], F8, tag="v")
                        for js in range(PB // P):
                            for kh in range(KH):
                                v_group(js, kh)
                        nc.sync.dma_start(ws_sb[:], r3(ws_d.ap()))
                        first = False
                    else:
                        if pb == 0:
                            # previous batch's last i-block PV+gate lands
                            # after the first K group, inside the new
                            # batch's projection stream
                            q_group(0)
                            k_group(0)
                            if deferred is not None:
                                flush(deferred)
                                deferred = None
                            v_sb = kvpool.tile([P, NJ, D], F8, tag="v")
                            for ec in range(1, DC):
                                q_group(ec)
                                k_group(ec)
                                v_group((ec - 1) // KH, (ec - 1) % KH)
                            for vg in range(DC - 1, DC):
                                v_group(vg // KH, vg % KH)
                        else:
                            # interleave Q/K/V so PSUM drains rotate over
                            # ACT/ACT/DVE (ps pool has only 4 bufs)
                            for ec in range(DC):
                                q_group(ec)
                                k_group(ec)
                                v_group(ec // KH, ec % KH)

                # ---- phase 2: attention + gate, per i-block ----
                for ib in range(NIB):
                    isl = slice(ib * IB, (ib + 1) * IB)
                    if ib == NIB - 1 and not last_batch:
                        # prefetch the next batch's first x tiles so its
                        # projections start without a DMA wait
                        nb = blist[bi + 1]
                        x1p = xin.tile([P, DC, PB], F8, tag="xin")
                        nc.sync.dma_start(x1p[:], r3(x1t_d[nb])[:, :, 0:PB])
                        x2p = xin.tile([P, DC, PB], F8, tag="xin")
                        nc.sync.dma_start(x2p[:], r3(x2t_d[nb])[:, :, 0:PB])
                        pre_x = (x1p, x2p)
                    p_all = pall_pool.tile([P, NJ, IB], BF16, tag="pall")
                    den_a = small.tile([P, IB], BF16, tag="dena")

                    # pass A: energy (fp8 DR) into 2-bank PSUM tiles so a
                    # single exp covers two j-tiles (exp bias/scale are
                    # constant, so tiles can share one ACT op -- halves the
                    # ACT op count); denominator accumulated in bf16 on DVE
                    # (2x mode)
                    for jh in range(NJ // 2):
                        ps2 = psE.tile([P, 2, IB], F32, tag="ps2")
                        for h in (0, 1):
                            j = 2 * jh + h
                            for dc2 in range(DC // 2):
                                mm8(ps2[:, h],
                                    kt_sb[:, 2 * dc2:2 * dc2 + 2,
                                          j * P:(j + 1) * P],
                                    qt_sb[:, 2 * dc2:2 * dc2 + 2, isl],
                                    start=(dc2 == 0),
                                    stop=(dc2 == DC // 2 - 1))
                        nc.scalar.activation(p_all[:, 2 * jh:2 * jh + 2, :],
                                             ps2[:], AF.Exp,
                                             bias=negshift[:, 0:1],
                                             scale=1.0 / (WSC * WSC))
                        for h in (0, 1):
                            j = 2 * jh + h
                            if j == 0:
                                nc.vector.tensor_copy(den_a[:],
                                                      p_all[:, j, :])
                            else:
                                nc.vector.tensor_tensor(den_a[:], den_a[:],
                                                        p_all[:, j, :],
                                                        OP.add)

                    # partition all-reduce on GpSimd (sum + broadcast in one
                    # op), then reciprocal on DVE.
                    den_all = small.tile([P, IB], F32, tag="denbf")
                    nc.gpsimd.partition_all_reduce(den_all[:], den_a[:], P,
                                                   ReduceOp.add)
                    rec_bc = small.tile([P, IB], F32, tag="recbc")
                    nc.vector.reciprocal(rec_bc[:], den_all[:])

                    final_ib = last_batch and ib == NIB - 1

                    def emit_norm(n_gp):
                        # normalize P and cast to fp8 (DVE first tiles,
                        # GpSimd the last -- PV consumes pairs in j order)
                        p_lo = p8_pool.tile([P, NJ, IB], F8, tag="p8")
                        for j in range(NJ - n_gp):
                            nc.vector.tensor_tensor(p_lo[:, j, :],
                                                    p_all[:, j, :],
                                                    rec_bc[:], OP.mult)
                        for j in range(NJ - n_gp, NJ):
                            nc.gpsimd.tensor_tensor(p_lo[:, j, :],
                                                    p_all[:, j, :],
                                                    rec_bc[:], OP.mult)
                        return p_lo

                    # PV+gate of the previous block runs here, hiding this
                    # block's denominator pipeline. For the very last block
                    # there is no following work to hide its softmax, so its
                    # norm is emitted first (ahead of the flush in the DVE
                    # queue) with a heavier GpSimd share.
                    if final_ib:
                        p_lo = emit_norm(8)
                        if deferred is not None:
                            flush(deferred)
                    else:
                        if deferred is not None:
                            flush(deferred)
                        # ib0 norm races PV(ib0) at the ib0->ib1 boundary
                        # (no flush spaces them), so give Pool less of it
                        n_gp = (6 if ib == 0 else 10) if NJ >= 16 else 0
                        p_lo = emit_norm(n_gp)

                    deferred = (p_lo, b, ib, v_sb)

            flush(deferred)

    nc.compile()
    return nc


def build_reps(BPC, N, D, gamma, reps=6):
    return build(BPC, N, D, gamma, reps=reps)


_CACHE = {}


def _get_nc(BPC, N, D, gamma):
    key = (BPC, N, D, float(gamma))
    if key not in _CACHE:
        _CACHE[key] = build(BPC, N, D, float(gamma))
    return _CACHE[key]


def make_in_maps(x1, x2, Wq, bq, Wk, bk, Wv, bv, W1, b1, W2, b2, W3, b3,
                 n_cores=N_CORES):
    """Host-side prep: shard over batch, transpose, cast fp8, fold weights."""
    f8 = ml_dtypes.float8_e4m3
    B, N, D = x1.shape
    DC = D // P
    Ws = (W1 + W2 + W3).astype(np.float32)
    bsum = (b1 + b2 + b3).astype(np.float32)

    def r_bias(v):  # [D] -> [128, DC] with v[c*128+p] at [p, c]
        return np.ascontiguousarray(v.reshape(DC, P).T).astype(np.float32)

    shared = {
        "wq_t": np.ascontiguousarray(WSC * Wq.T).astype(f8),
        "wk_t": np.ascontiguousarray(WSC * Wk.T).astype(f8),
        "wv_t": np.ascontiguousarray(WSC * Wv.T).astype(f8),
        "ws_t": np.ascontiguousarray(WSC * Ws.T).astype(f8),
        "b_all": np.ascontiguousarray(np.stack(
            [r_bias(WSC * bq), r_bias(WSC * bk),
             r_bias(bv), r_bias(0.5 * bsum)], axis=1)),
    }
    bpc = B // n_cores
    in_maps = []
    for c in range(n_cores):
        sl = slice(c * bpc, (c + 1) * bpc)
        in_maps.append({
            "x1t_8": np.ascontiguousarray(x1[sl].transpose(0, 2, 1)).astype(f8),
            "x2t_8": np.ascontiguousarray(x2[sl].transpose(0, 2, 1)).astype(f8),
            **shared,
        })
    return in_maps


def kernel(x1, x2, Wq, bq, Wk, bk, Wv, bv, W1, b1, W2, b2, W3, b3, gamma):
    from concourse.bass_utils import run_bass_kernel_spmd

    x1 = np.asarray(x1, dtype=np.float32)
    x2 = np.asarray(x2, dtype=np.float32)
    B, N, D = x1.shape
    bpc = B // N_CORES
    nc = _get_nc(bpc, N, D, float(np.asarray(gamma).reshape(-1)[0]))
    in_maps = make_in_maps(x1, x2, np.asarray(Wq), np.asarray(bq),
                           np.asarray(Wk), np.asarray(bk),
                           np.asarray(Wv), np.asarray(bv),
                           np.asarray(W1), np.asarray(b1),
                           np.asarray(W2), np.asarray(b2),
                           np.asarray(W3), np.asarray(b3))
    out = np.empty((B, N, D), np.float32)
    # transient axon/NRT glitches occasionally corrupt a run (non-finite
    # values); the kernel itself is deterministic, so retry on detection
    for attempt in range(3):
        res = run_bass_kernel_spmd(nc, in_maps, list(range(N_CORES)))
        for c in range(N_CORES):
            out[c * bpc:(c + 1) * bpc] = \
                res.results[c]["out"].astype(np.float32).transpose(0, 2, 1)
        if np.isfinite(out).all():
            break
    # device computed fin = (tanh((s+bs)/2) + 1) * out; the gamma/2 factor
    # and the residual are applied here
    g2 = 0.5 * float(np.asarray(gamma).reshape(-1)[0])
    out = out * g2 + x1
    return out
